# revision 22
# baseline (speedup 1.0000x reference)
"""MultiLabelContrastiveLoss Trainium2 kernel (8 NeuronCores, Bass/Tile).

Math (reference):
    sim = (emb @ emb.T) / T                      # [B, B]
    cnt[i,j] = #aspects where labels match       # via one-hot GEMM
    positive_mask = (cnt/A >= 0.5) & offdiag
    pos_i = sum_j exp(sim) * positive_mask
    all_i = sum_j exp(sim) * offdiag
    valid = pos > 0
    loss = sum(valid * -log(where(valid,pos,1)/(all+eps))) / max(n_valid, 1)

Kernel strategy (per core, SPMD over 8 cores):
  - Each core owns a 1024-row block of the sim matrix; its inputs are
    column-ROTATED so its own block is always local columns [0, 1024).
    This makes diagonal positions compile-time constants under SPMD, and
    row-sums are invariant to column order.
  - Per [128 x 1024] tile group:
      PE:  sim_psum  = embT_rows.T @ embT_cols          (fp32, K=128)
      PE:  cnt_psum  = ohL.T @ ohR                      (bf16, K=37)
           where oh rows 0..35 are the one-hot(label) rows and row 36 is
           the augment (lhs=1, rhs=-5.5) so cnt_psum = cnt - 5.5 exactly;
           mask test becomes (cnt_psum >= 0).
      DVE: diagonal tiles only: sim_psum[diag] += -1e30  (exact exclusion)
      ACT: E = exp(sim_psum / T) -> SBUF, accum_out -> allsim partial
      DVE: scalar_tensor_tensor: (cnt_psum >= 0) * E, accum_out -> pos partial
  - Tiny epilogue per core -> per-core partials [1, 2] = (sum_contrib, n_valid).
  - Host gathers the 8 partial pairs: loss = sum_c / max(sum_v, 1).
"""

import numpy as np
import ml_dtypes

import concourse.bass as bass
import concourse.tile as tile
from concourse import bacc, mybir
from concourse.bass_utils import run_bass_kernel_spmd

F32 = mybir.dt.float32
BF16 = mybir.dt.bfloat16

B = 8192          # batch
D = 128           # embedding dim
A = 12            # aspects
NCLS = 3          # classes
TEMP = 0.07
EPS = 1e-8
NEG = -1.0e30
N_CORES = 8
G = 1024          # column group width (2 PSUM banks fp32)
# one-hot GEMM layout: class c's 12 rows live at partitions [32c, 32c+12)
# (engine base partitions must be 32-aligned), augment row at partition 96,
# all gap rows zeroed -> contraction K = 97.
KOH = 97


def build(b=B, n_cores=N_CORES, debug_outputs=False):
    """Build + compile the SPMD kernel module for batch size b."""
    rpc = b // n_cores        # rows per core
    rt = rpc // 128           # 128-row tiles per core
    ng = b // G               # column groups per row tile
    nacc = rt * ng

    nc = bacc.Bacc("TRN2", target_bir_lowering=False, debug=False,
                   num_devices=n_cores)
    embT_d = nc.dram_tensor("embT", [D, b], BF16, kind="ExternalInput")
    # labels transposed, padded to 32 partitions with -1 filler rows, so a
    # 32-row is_equal writes the one-hot class block AND zero gap rows in one op
    labT_d = nc.dram_tensor("labT", [32, b], BF16, kind="ExternalInput")
    ident_d = nc.dram_tensor("identb", [128, 128], BF16, kind="ExternalInput")
    dneg_d = nc.dram_tensor("diagnegb", [128, 128], BF16, kind="ExternalInput")
    out_d = nc.dram_tensor("out", [1, 2], F32, kind="ExternalOutput")
    if debug_outputs:
        posr_d = nc.dram_tensor("pos_r", [128, rt], F32, kind="ExternalOutput")
        allr_d = nc.dram_tensor("all_r", [128, rt], F32, kind="ExternalOutput")

    with tile.TileContext(nc) as tc:
        with (
            tc.tile_pool(name="const", bufs=1) as cpool,
            tc.tile_pool(name="ework", bufs=3) as epool,
            tc.tile_pool(name="junk", bufs=2) as jpool,
            tc.tile_pool(name="psA", bufs=2, space="PSUM") as psA,
            tc.tile_pool(name="psB", bufs=2, space="PSUM") as psB,
        ):
            emb_sb = cpool.tile([D, b], BF16)
            lab_sb = cpool.tile([32, b], BF16)
            ident_sb = cpool.tile([128, 128], BF16)
            dneg_sb = cpool.tile([128, 128], BF16)
            ohL = cpool.tile([KOH, rpc], BF16)
            ohR = cpool.tile([KOH, b], BF16)
            pos_acc = cpool.tile([128, nacc], F32)
            all_acc = cpool.tile([128, nacc], F32)

            # ---- input DMAs (chunked for multi-queue overlap) ----
            nchunk = max(1, b // 512)
            for k in range(nchunk):
                sl = slice(k * 512, (k + 1) * 512)
                nc.sync.dma_start(emb_sb[:, sl], embT_d[:, sl])
            nc.sync.dma_start(lab_sb[:], labT_d[:])
            nc.sync.dma_start(ident_sb[:], ident_d[:])
            nc.sync.dma_start(dneg_sb[:], dneg_d[:])

            # ---- one-hot build (bf16, 4x mode on DVE) ----
            # Augment rows first (no input deps; gpsimd is otherwise idle).
            nc.gpsimd.memset(ohR[96:97, :], -(A / 2.0 - 0.5))  # -5.5
            nc.gpsimd.memset(ohL[96:97, :], 1.0)
            # 32-row is_equal per class: rows 32c..32c+11 get the one-hot,
            # rows 32c+12..32c+31 read the -1 filler -> exact zeros.
            for c in range(NCLS):
                nc.vector.tensor_scalar(
                    out=ohL[32 * c:32 * c + 32, :],
                    in0=lab_sb[:, 0:rpc],
                    scalar1=float(c), scalar2=None,
                    op0=mybir.AluOpType.is_equal,
                )
            # Leading G columns first (unblocks group 0), then the rest.
            pieces = [(0, min(G, b))]
            if b > G:
                pieces.append((G, b))
            for lo, hi in pieces:
                for c in range(NCLS):
                    nc.vector.tensor_scalar(
                        out=ohR[32 * c:32 * c + 32, lo:hi],
                        in0=lab_sb[:, lo:hi],
                        scalar1=float(c), scalar2=None,
                        op0=mybir.AluOpType.is_equal,
                    )

            # ---- main loop ----
            for r in range(rt):
                rsl = slice(128 * r, 128 * r + 128)
                for g in range(ng):
                    sim_ps = psA.tile([128, G], F32, tag="sim")
                    cnt_ps = psB.tile([128, G], F32, tag="cnt")
                    # diagonal lands in this group? (rotation => local col 128r)
                    has_diag = G * g <= 128 * r < G * (g + 1)
                    dloc = 128 * r - G * g
                    for h in range(G // 512):
                        csl = slice(G * g + 512 * h, G * g + 512 * (h + 1))
                        osl = slice(512 * h, 512 * (h + 1))
                        dh = has_diag and 512 * h <= dloc < 512 * (h + 1)
                        nc.tensor.matmul(sim_ps[:, osl], emb_sb[:, rsl],
                                         emb_sb[:, csl], start=True,
                                         stop=not dh)
                        if dh:
                            # exact diag exclusion: accumulate -1e30*I on PE
                            dsl = slice(dloc, dloc + 128)
                            nc.tensor.matmul(sim_ps[:, dsl], ident_sb[:],
                                             dneg_sb[:], start=False, stop=True)
                        nc.tensor.matmul(cnt_ps[:, osl], ohL[:, rsl],
                                         ohR[:, csl], start=True, stop=True)
                    idx = ng * r + g
                    e_t = epool.tile([128, G], F32, tag="E")
                    nc.scalar.activation(
                        e_t[:], sim_ps[:], mybir.ActivationFunctionType.Exp,
                        scale=1.0 / TEMP,
                        accum_out=all_acc[:, idx:idx + 1],
                    )
                    junk = jpool.tile([128, G], F32, tag="junk")
                    nc.vector.scalar_tensor_tensor(
                        out=junk[:],
                        in0=cnt_ps[:], scalar=0.0, in1=e_t[:],
                        op0=mybir.AluOpType.is_ge,
                        op1=mybir.AluOpType.mult,
                        accum_out=pos_acc[:, idx:idx + 1],
                    )

            # ---- epilogue: per-core partials ----
            pos_r = cpool.tile([128, rt], F32)
            all_r = cpool.tile([128, rt], F32)
            nc.vector.reduce_sum(
                pos_r[:], pos_acc[:].rearrange("p (r g) -> p r g", g=ng),
                axis=mybir.AxisListType.X)
            nc.vector.reduce_sum(
                all_r[:], all_acc[:].rearrange("p (r g) -> p r g", g=ng),
                axis=mybir.AxisListType.X)

            valid = cpool.tile([128, rt], F32)
            p1 = cpool.tile([128, rt], F32)
            la = cpool.tile([128, rt], F32)
            lp = cpool.tile([128, rt], F32)
            contrib = cpool.tile([128, rt], F32)
            cv = cpool.tile([128, 2], F32)
            ones = cpool.tile([128, 1], F32)
            eps_t = cpool.tile([128, 1], F32)
            tot_sb = cpool.tile([1, 2], F32)
            nc.vector.memset(eps_t[:], EPS)

            nc.vector.tensor_scalar(out=valid[:], in0=pos_r[:], scalar1=0.0,
                                    scalar2=None, op0=mybir.AluOpType.is_gt)
            # p1 = pos + (pos <= 0)  -> pos for valid rows, 1 for invalid
            nc.vector.scalar_tensor_tensor(
                out=p1[:], in0=pos_r[:], scalar=0.0, in1=pos_r[:],
                op0=mybir.AluOpType.is_le, op1=mybir.AluOpType.add)
            nc.scalar.activation(la[:], all_r[:],
                                 mybir.ActivationFunctionType.Ln, bias=eps_t[:])
            nc.scalar.activation(lp[:], p1[:],
                                 mybir.ActivationFunctionType.Ln)
            # contrib = valid*la - lp   (lp = 0 for invalid rows)
            nc.vector.tensor_mul(contrib[:], valid[:], la[:])
            nc.vector.tensor_sub(contrib[:], contrib[:], lp[:])

            nc.vector.reduce_sum(cv[:, 0:1], contrib[:], axis=mybir.AxisListType.X)
            nc.vector.reduce_sum(cv[:, 1:2], valid[:], axis=mybir.AxisListType.X)

            nc.vector.memset(ones[:], 1.0)
            tot_ps = psA.tile([1, 2], F32, tag="sim")
            nc.tensor.matmul(tot_ps[:], ones[:], cv[:], start=True, stop=True)
            nc.vector.tensor_copy(tot_sb[:], tot_ps[:])
            nc.sync.dma_start(out_d[:], tot_sb[:])
            if debug_outputs:
                nc.sync.dma_start(posr_d[:], pos_r[:])
                nc.sync.dma_start(allr_d[:], all_r[:])

    nc.compile()
    return nc


_CACHE = {}


def _get_nc(b=B, n_cores=N_CORES):
    key = (b, n_cores)
    if key not in _CACHE:
        _CACHE[key] = build(b, n_cores)
    return _CACHE[key]


def make_in_maps(embeddings, labels, b=B, n_cores=N_CORES):
    embT = np.ascontiguousarray(
        embeddings.astype(np.float32).T).astype(ml_dtypes.bfloat16)  # [D, b]
    labT = np.full((32, b), -1.0, dtype=np.float32)
    labT[:A] = np.asarray(labels).astype(np.float32).T
    labT = labT.astype(ml_dtypes.bfloat16)
    ident = np.eye(128, dtype=np.float32).astype(ml_dtypes.bfloat16)
    dneg = (np.eye(128, dtype=np.float32) * NEG).astype(ml_dtypes.bfloat16)
    rpc = b // n_cores
    in_maps = []
    for c in range(n_cores):
        s = c * rpc
        emb_rot = np.ascontiguousarray(
            np.concatenate([embT[:, s:], embT[:, :s]], axis=1))
        lab_rot = np.ascontiguousarray(
            np.concatenate([labT[:, s:], labT[:, :s]], axis=1))
        in_maps.append({"embT": emb_rot, "labT": lab_rot,
                        "identb": ident, "diagnegb": dneg})
    return in_maps


def combine_partials(parts):
    """parts: list of [1,2] arrays per core -> scalar loss (reference math)."""
    tot = np.sum(np.stack([p.reshape(2) for p in parts]), axis=0,
                 dtype=np.float64)
    c, v = tot[0], tot[1]
    loss = c / max(v, 1.0) if v > 0 else 0.0
    return np.array(loss, dtype=np.float32)


def kernel(embeddings, labels):
    nc = _get_nc(B, N_CORES)
    in_maps = make_in_maps(embeddings, labels, B, N_CORES)
    res = run_bass_kernel_spmd(nc, in_maps, core_ids=list(range(N_CORES)))
    return combine_partials([r["out"] for r in res.results])


# revision 23
# speedup vs baseline: 1.0928x; 1.0928x over previous
"""MultiLabelContrastiveLoss Trainium2 kernel (8 NeuronCores, Bass/Tile).

Math (reference):
    sim = (emb @ emb.T) / T                      # [B, B]
    cnt[i,j] = #aspects where labels match       # via one-hot GEMM
    positive_mask = (cnt/A >= 0.5) & offdiag
    pos_i = sum_j exp(sim) * positive_mask
    all_i = sum_j exp(sim) * offdiag
    valid = pos > 0
    loss = sum(valid * -log(where(valid,pos,1)/(all+eps))) / max(n_valid, 1)

Kernel strategy (per core, SPMD over 8 cores):
  - Each core owns a 1024-row block of the sim matrix; its inputs are
    column-ROTATED so its own block is always local columns [0, 1024).
    This makes diagonal positions compile-time constants under SPMD, and
    row-sums are invariant to column order.
  - Per [128 x 1024] tile group:
      PE:  sim_psum  = embT_rows.T @ embT_cols          (fp32, K=128)
      PE:  cnt_psum  = ohL.T @ ohR                      (bf16, K=37)
           where oh rows 0..35 are the one-hot(label) rows and row 36 is
           the augment (lhs=1, rhs=-5.5) so cnt_psum = cnt - 5.5 exactly;
           mask test becomes (cnt_psum >= 0).
      DVE: diagonal tiles only: sim_psum[diag] += -1e30  (exact exclusion)
      ACT: E = exp(sim_psum / T) -> SBUF, accum_out -> allsim partial
      DVE: scalar_tensor_tensor: (cnt_psum >= 0) * E, accum_out -> pos partial
  - Tiny epilogue per core -> per-core partials [1, 2] = (sum_contrib, n_valid).
  - Host gathers the 8 partial pairs: loss = sum_c / max(sum_v, 1).
"""

import numpy as np
import ml_dtypes

import concourse.bass as bass
import concourse.tile as tile
from concourse import bacc, mybir
from concourse.bass_utils import run_bass_kernel_spmd

F32 = mybir.dt.float32
BF16 = mybir.dt.bfloat16

B = 8192          # batch
D = 128           # embedding dim
A = 12            # aspects
NCLS = 3          # classes
TEMP = 0.07
EPS = 1e-8
NEG = -1.0e30
N_CORES = 8
G = 1024          # column group width (2 PSUM banks fp32)
# one-hot GEMM layout: class c's 12 rows live at partitions [32c, 32c+12)
# (engine base partitions must be 32-aligned), augment row at partition 96,
# all gap rows zeroed -> contraction K = 97.
KOH = 97


def build(b=B, n_cores=N_CORES, debug_outputs=False):
    """Build + compile the SPMD kernel module for batch size b."""
    rpc = b // n_cores        # rows per core
    rt = rpc // 128           # 128-row tiles per core
    ng = b // G               # column groups per row tile
    nacc = rt * ng

    nc = bacc.Bacc("TRN2", target_bir_lowering=False, debug=False,
                   num_devices=n_cores)
    embT_d = nc.dram_tensor("embT", [D, b], BF16, kind="ExternalInput")
    # labels transposed, padded to 32 partitions with -1 filler rows, so a
    # 32-row is_equal writes the one-hot class block AND zero gap rows in one op
    labT_d = nc.dram_tensor("labT", [32, b], BF16, kind="ExternalInput")
    ident_d = nc.dram_tensor("identb", [128, 128], BF16, kind="ExternalInput")
    dneg_d = nc.dram_tensor("diagnegb", [128, 128], BF16, kind="ExternalInput")
    out_d = nc.dram_tensor("out", [1, 2], F32, kind="ExternalOutput")
    if debug_outputs:
        posr_d = nc.dram_tensor("pos_r", [128, rt], F32, kind="ExternalOutput")
        allr_d = nc.dram_tensor("all_r", [128, rt], F32, kind="ExternalOutput")

    with tile.TileContext(nc) as tc:
        with (
            tc.tile_pool(name="const", bufs=1) as cpool,
            tc.tile_pool(name="ework", bufs=3) as epool,
            tc.tile_pool(name="junk", bufs=2) as jpool,
            tc.tile_pool(name="psA", bufs=2, space="PSUM") as psA,
            tc.tile_pool(name="psB", bufs=2, space="PSUM") as psB,
        ):
            emb_sb = cpool.tile([D, b], BF16)
            lab_sb = cpool.tile([32, b], BF16)
            ident_sb = cpool.tile([128, 128], BF16)
            dneg_sb = cpool.tile([128, 128], BF16)
            ohL = cpool.tile([KOH, rpc], BF16)
            ohR = cpool.tile([KOH, b], BF16)
            pos_acc = cpool.tile([128, nacc], F32)
            all_acc = cpool.tile([128, nacc], F32)

            # ---- input DMAs: small blockers first, then emb chunks in
            # consumption order ----
            nc.sync.dma_start(lab_sb[:], labT_d[:])
            nc.sync.dma_start(ident_sb[:], ident_d[:])
            nc.sync.dma_start(dneg_sb[:], dneg_d[:])
            nchunk = max(1, b // 1024)
            for k in range(nchunk):
                sl = slice(k * 1024, (k + 1) * 1024)
                nc.sync.dma_start(emb_sb[:, sl], embT_d[:, sl])

            # ---- one-hot build (bf16, 4x mode on DVE) ----
            # Augment rows first (no input deps; gpsimd is otherwise idle).
            nc.gpsimd.memset(ohR[96:97, :], -(A / 2.0 - 0.5))  # -5.5
            nc.gpsimd.memset(ohL[96:97, :], 1.0)
            # 32-row is_equal per class: rows 32c..32c+11 get the one-hot,
            # rows 32c+12..32c+31 read the -1 filler -> exact zeros.
            for c in range(NCLS):
                nc.vector.tensor_scalar(
                    out=ohL[32 * c:32 * c + 32, :],
                    in0=lab_sb[:, 0:rpc],
                    scalar1=float(c), scalar2=None,
                    op0=mybir.AluOpType.is_equal,
                )
            # Leading G columns first (unblocks group 0), then the rest.
            pieces = [(0, min(G, b))]
            if b > G:
                pieces.append((G, b))
            for lo, hi in pieces:
                for c in range(NCLS):
                    nc.vector.tensor_scalar(
                        out=ohR[32 * c:32 * c + 32, lo:hi],
                        in0=lab_sb[:, lo:hi],
                        scalar1=float(c), scalar2=None,
                        op0=mybir.AluOpType.is_equal,
                    )

            # ---- main loop ----
            for r in range(rt):
                rsl = slice(128 * r, 128 * r + 128)
                for g in range(ng):
                    sim_ps = psA.tile([128, G], F32, tag="sim")
                    cnt_ps = psB.tile([128, G], F32, tag="cnt")
                    # diagonal lands in this group? (rotation => local col 128r)
                    has_diag = G * g <= 128 * r < G * (g + 1)
                    dloc = 128 * r - G * g
                    for h in range(G // 512):
                        csl = slice(G * g + 512 * h, G * g + 512 * (h + 1))
                        osl = slice(512 * h, 512 * (h + 1))
                        dh = has_diag and 512 * h <= dloc < 512 * (h + 1)
                        nc.tensor.matmul(sim_ps[:, osl], emb_sb[:, rsl],
                                         emb_sb[:, csl], start=True,
                                         stop=not dh)
                        if dh:
                            # exact diag exclusion: accumulate -1e30*I on PE
                            dsl = slice(dloc, dloc + 128)
                            nc.tensor.matmul(sim_ps[:, dsl], ident_sb[:],
                                             dneg_sb[:], start=False, stop=True)
                        nc.tensor.matmul(cnt_ps[:, osl], ohL[:, rsl],
                                         ohR[:, csl], start=True, stop=True)
                    idx = ng * r + g
                    e_t = epool.tile([128, G], F32, tag="E")
                    nc.scalar.activation(
                        e_t[:], sim_ps[:], mybir.ActivationFunctionType.Exp,
                        scale=1.0 / TEMP,
                        accum_out=all_acc[:, idx:idx + 1],
                    )
                    junk = jpool.tile([128, G], F32, tag="junk")
                    nc.vector.scalar_tensor_tensor(
                        out=junk[:],
                        in0=cnt_ps[:], scalar=0.0, in1=e_t[:],
                        op0=mybir.AluOpType.is_ge,
                        op1=mybir.AluOpType.mult,
                        accum_out=pos_acc[:, idx:idx + 1],
                    )

            # ---- epilogue: per-core partials ----
            pos_r = cpool.tile([128, rt], F32)
            all_r = cpool.tile([128, rt], F32)
            nc.vector.reduce_sum(
                pos_r[:], pos_acc[:].rearrange("p (r g) -> p r g", g=ng),
                axis=mybir.AxisListType.X)
            nc.vector.reduce_sum(
                all_r[:], all_acc[:].rearrange("p (r g) -> p r g", g=ng),
                axis=mybir.AxisListType.X)

            valid = cpool.tile([128, rt], F32)
            p1 = cpool.tile([128, rt], F32)
            la = cpool.tile([128, rt], F32)
            lp = cpool.tile([128, rt], F32)
            contrib = cpool.tile([128, rt], F32)
            cv = cpool.tile([128, 2], F32)
            ones = cpool.tile([128, 1], F32)
            eps_t = cpool.tile([128, 1], F32)
            tot_sb = cpool.tile([1, 2], F32)
            nc.vector.memset(eps_t[:], EPS)

            nc.vector.tensor_scalar(out=valid[:], in0=pos_r[:], scalar1=0.0,
                                    scalar2=None, op0=mybir.AluOpType.is_gt)
            # p1 = pos + (pos <= 0)  -> pos for valid rows, 1 for invalid
            nc.vector.scalar_tensor_tensor(
                out=p1[:], in0=pos_r[:], scalar=0.0, in1=pos_r[:],
                op0=mybir.AluOpType.is_le, op1=mybir.AluOpType.add)
            nc.scalar.activation(la[:], all_r[:],
                                 mybir.ActivationFunctionType.Ln, bias=eps_t[:])
            nc.scalar.activation(lp[:], p1[:],
                                 mybir.ActivationFunctionType.Ln)
            # contrib = valid*la - lp   (lp = 0 for invalid rows)
            nc.vector.tensor_mul(contrib[:], valid[:], la[:])
            nc.vector.tensor_sub(contrib[:], contrib[:], lp[:])

            nc.vector.reduce_sum(cv[:, 0:1], contrib[:], axis=mybir.AxisListType.X)
            nc.vector.reduce_sum(cv[:, 1:2], valid[:], axis=mybir.AxisListType.X)

            nc.vector.memset(ones[:], 1.0)
            tot_ps = psA.tile([1, 2], F32, tag="sim")
            nc.tensor.matmul(tot_ps[:], ones[:], cv[:], start=True, stop=True)
            nc.vector.tensor_copy(tot_sb[:], tot_ps[:])
            nc.sync.dma_start(out_d[:], tot_sb[:])
            if debug_outputs:
                nc.sync.dma_start(posr_d[:], pos_r[:])
                nc.sync.dma_start(allr_d[:], all_r[:])

    nc.compile()
    return nc


_CACHE = {}


def _get_nc(b=B, n_cores=N_CORES):
    key = (b, n_cores)
    if key not in _CACHE:
        _CACHE[key] = build(b, n_cores)
    return _CACHE[key]


def make_in_maps(embeddings, labels, b=B, n_cores=N_CORES):
    embT = np.ascontiguousarray(
        embeddings.astype(np.float32).T).astype(ml_dtypes.bfloat16)  # [D, b]
    labT = np.full((32, b), -1.0, dtype=np.float32)
    labT[:A] = np.asarray(labels).astype(np.float32).T
    labT = labT.astype(ml_dtypes.bfloat16)
    ident = np.eye(128, dtype=np.float32).astype(ml_dtypes.bfloat16)
    dneg = (np.eye(128, dtype=np.float32) * NEG).astype(ml_dtypes.bfloat16)
    rpc = b // n_cores
    in_maps = []
    for c in range(n_cores):
        s = c * rpc
        emb_rot = np.ascontiguousarray(
            np.concatenate([embT[:, s:], embT[:, :s]], axis=1))
        lab_rot = np.ascontiguousarray(
            np.concatenate([labT[:, s:], labT[:, :s]], axis=1))
        in_maps.append({"embT": emb_rot, "labT": lab_rot,
                        "identb": ident, "diagnegb": dneg})
    return in_maps


def combine_partials(parts):
    """parts: list of [1,2] arrays per core -> scalar loss (reference math)."""
    tot = np.sum(np.stack([p.reshape(2) for p in parts]), axis=0,
                 dtype=np.float64)
    c, v = tot[0], tot[1]
    loss = c / max(v, 1.0) if v > 0 else 0.0
    return np.array(loss, dtype=np.float32)


def kernel(embeddings, labels):
    nc = _get_nc(B, N_CORES)
    in_maps = make_in_maps(embeddings, labels, B, N_CORES)
    res = run_bass_kernel_spmd(nc, in_maps, core_ids=list(range(N_CORES)))
    return combine_partials([r["out"] for r in res.results])


# revision 30
# speedup vs baseline: 1.1307x; 1.0347x over previous
"""MultiLabelContrastiveLoss Trainium2 kernel (8 NeuronCores, Bass/Tile).

Math (reference):
    sim = (emb @ emb.T) / T                      # [B, B]
    cnt[i,j] = #aspects where labels match       # via one-hot GEMM
    positive_mask = (cnt/A >= 0.5) & offdiag
    pos_i = sum_j exp(sim) * positive_mask
    all_i = sum_j exp(sim) * offdiag
    valid = pos > 0
    loss = sum(valid * -log(where(valid,pos,1)/(all+eps))) / max(n_valid, 1)

Kernel strategy (per core, SPMD over 8 cores):
  - Each core owns a 1024-row block of the sim matrix; its inputs are
    column-ROTATED so its own block is always local columns [0, 1024).
    This makes diagonal positions compile-time constants under SPMD, and
    row-sums are invariant to column order.
  - Per [128 x 1024] tile group:
      PE:  sim_psum  = embT_rows.T @ embT_cols          (fp32, K=128)
      PE:  cnt_psum  = ohL.T @ ohR                      (bf16, K=37)
           where oh rows 0..35 are the one-hot(label) rows and row 36 is
           the augment (lhs=1, rhs=-5.5) so cnt_psum = cnt - 5.5 exactly;
           mask test becomes (cnt_psum >= 0).
      DVE: diagonal tiles only: sim_psum[diag] += -1e30  (exact exclusion)
      ACT: E = exp(sim_psum / T) -> SBUF, accum_out -> allsim partial
      DVE: scalar_tensor_tensor: (cnt_psum >= 0) * E, accum_out -> pos partial
  - Tiny epilogue per core -> per-core partials [1, 2] = (sum_contrib, n_valid).
  - Host gathers the 8 partial pairs: loss = sum_c / max(sum_v, 1).
"""

import numpy as np
import ml_dtypes

import concourse.bass as bass
import concourse.tile as tile
from concourse import bacc, mybir
from concourse.bass_utils import run_bass_kernel_spmd

F32 = mybir.dt.float32
BF16 = mybir.dt.bfloat16

B = 8192          # batch
D = 128           # embedding dim
A = 12            # aspects
NCLS = 3          # classes
TEMP = 0.07
EPS = 1e-8
NEG = -1.0e30
N_CORES = 8
G = 1024          # column group width (2 PSUM banks fp32)
# one-hot GEMM layout: class c's 12 rows live at partitions [32c, 32c+12)
# (engine base partitions must be 32-aligned), augment row at partition 96,
# all gap rows zeroed -> contraction K = 97.
KOH = 97


def build(b=B, n_cores=N_CORES, debug_outputs=False):
    """Build + compile the SPMD kernel module for batch size b."""
    rpc = b // n_cores        # rows per core
    rt = rpc // 128           # 128-row tiles per core
    ng = b // G               # column groups per row tile
    nacc = rt * ng

    nc = bacc.Bacc("TRN2", target_bir_lowering=False, debug=False,
                   num_devices=n_cores)
    embT_d = nc.dram_tensor("embT", [D, b], BF16, kind="ExternalInput")
    # labels transposed, padded to 32 partitions with -1 filler rows, so a
    # 32-row is_equal writes the one-hot class block AND zero gap rows in one op
    labT_d = nc.dram_tensor("labT", [32, b], BF16, kind="ExternalInput")
    # augment rows for the one-hot GEMM (row0: -5.5 for rhs, row1: 1.0 for lhs)
    aug_d = nc.dram_tensor("aug", [2, b], BF16, kind="ExternalInput")
    ident_d = nc.dram_tensor("identb", [128, 128], BF16, kind="ExternalInput")
    dneg_d = nc.dram_tensor("diagnegb", [128, 128], BF16, kind="ExternalInput")
    out_d = nc.dram_tensor("out", [1, 2], F32, kind="ExternalOutput")
    if debug_outputs:
        posr_d = nc.dram_tensor("pos_r", [128, rt], F32, kind="ExternalOutput")
        allr_d = nc.dram_tensor("all_r", [128, rt], F32, kind="ExternalOutput")

    with tile.TileContext(nc) as tc:
        with (
            tc.tile_pool(name="const", bufs=1) as cpool,
            tc.tile_pool(name="ework", bufs=3) as epool,
            tc.tile_pool(name="junk", bufs=2) as jpool,
            tc.tile_pool(name="psA", bufs=2, space="PSUM") as psA,
            tc.tile_pool(name="psB", bufs=2, space="PSUM") as psB,
        ):
            emb_sb = cpool.tile([D, b], BF16)
            lab_sb = cpool.tile([32, b], BF16)
            ident_sb = cpool.tile([128, 128], BF16)
            dneg_sb = cpool.tile([128, 128], BF16)
            ohL = cpool.tile([KOH, rpc], BF16)
            ohR = cpool.tile([KOH, b], BF16)
            pos_acc = cpool.tile([128, nacc], F32)
            all_acc = cpool.tile([128, nacc], F32)

            # ---- input DMAs: small blockers first, then emb chunks in
            # consumption order ----
            nc.sync.dma_start(lab_sb[:, 0:G], labT_d[:, 0:G])
            nc.sync.dma_start(ohR[96:97, :], aug_d[0:1, :])
            nc.sync.dma_start(ohL[96:97, :], aug_d[1:2, 0:rpc])
            nc.sync.dma_start(ident_sb[:], ident_d[:])
            nc.sync.dma_start(dneg_sb[:], dneg_d[:])
            if b > G:
                nc.sync.dma_start(lab_sb[:, G:b], labT_d[:, G:b])
            nchunk = max(1, b // 1024)
            for k in range(nchunk):
                sl = slice(k * 1024, (k + 1) * 1024)
                nc.sync.dma_start(emb_sb[:, sl], embT_d[:, sl])

            # ---- one-hot build (bf16, 4x mode on DVE) ----
            # 32-row is_equal per class: rows 32c..32c+11 get the one-hot,
            # rows 32c+12..32c+31 read the -1 filler -> exact zeros.
            # Leading G columns first (unblocks group 0), then ohL, then rest.
            pieces = [(0, min(G, b))]
            if b > G:
                pieces.append((G, b))
            for c in range(NCLS):
                nc.vector.tensor_scalar(
                    out=ohR[32 * c:32 * c + 32, 0:pieces[0][1]],
                    in0=lab_sb[:, 0:pieces[0][1]],
                    scalar1=float(c), scalar2=None,
                    op0=mybir.AluOpType.is_equal,
                )
            for c in range(NCLS):
                nc.vector.tensor_scalar(
                    out=ohL[32 * c:32 * c + 32, :],
                    in0=lab_sb[:, 0:rpc],
                    scalar1=float(c), scalar2=None,
                    op0=mybir.AluOpType.is_equal,
                )
            if b > G:
                for c in range(NCLS):
                    nc.vector.tensor_scalar(
                        out=ohR[32 * c:32 * c + 32, G:b],
                        in0=lab_sb[:, G:b],
                        scalar1=float(c), scalar2=None,
                        op0=mybir.AluOpType.is_equal,
                    )

            # preload the Ln activation table so the epilogue doesn't pay
            # a table switch on the critical tail
            eps_t = cpool.tile([128, 1], F32)
            lnwarm = cpool.tile([128, 1], F32)
            nc.vector.memset(eps_t[:], EPS)
            nc.scalar.activation(lnwarm[:], eps_t[:],
                                 mybir.ActivationFunctionType.Ln)

            # ---- main loop ----
            for r in range(rt):
                rsl = slice(128 * r, 128 * r + 128)
                for g in range(ng):
                    sim_ps = psA.tile([128, G], F32, tag="sim")
                    cnt_ps = psB.tile([128, G], F32, tag="cnt")
                    # diagonal lands in this group? (rotation => local col 128r)
                    has_diag = G * g <= 128 * r < G * (g + 1)
                    dloc = 128 * r - G * g
                    for h in range(G // 512):
                        csl = slice(G * g + 512 * h, G * g + 512 * (h + 1))
                        osl = slice(512 * h, 512 * (h + 1))
                        dh = has_diag and 512 * h <= dloc < 512 * (h + 1)
                        nc.tensor.matmul(sim_ps[:, osl], emb_sb[:, rsl],
                                         emb_sb[:, csl], start=True,
                                         stop=not dh)
                        if dh:
                            # exact diag exclusion: accumulate -1e30*I on PE
                            dsl = slice(dloc, dloc + 128)
                            nc.tensor.matmul(sim_ps[:, dsl], ident_sb[:],
                                             dneg_sb[:], start=False, stop=True)
                        nc.tensor.matmul(cnt_ps[:, osl], ohL[:, rsl],
                                         ohR[:, csl], start=True, stop=True)
                    idx = ng * r + g
                    e_t = epool.tile([128, G], F32, tag="E")
                    nc.scalar.activation(
                        e_t[:], sim_ps[:], mybir.ActivationFunctionType.Exp,
                        scale=1.0 / TEMP,
                        accum_out=all_acc[:, idx:idx + 1],
                    )
                    junk = jpool.tile([128, G], F32, tag="junk")
                    nc.vector.scalar_tensor_tensor(
                        out=junk[:],
                        in0=cnt_ps[:], scalar=0.0, in1=e_t[:],
                        op0=mybir.AluOpType.is_ge,
                        op1=mybir.AluOpType.mult,
                        accum_out=pos_acc[:, idx:idx + 1],
                    )

            # ---- epilogue: per-core partials ----
            pos_r = cpool.tile([128, rt], F32)
            all_r = cpool.tile([128, rt], F32)
            nc.vector.reduce_sum(
                pos_r[:], pos_acc[:].rearrange("p (r g) -> p r g", g=ng),
                axis=mybir.AxisListType.X)
            nc.vector.reduce_sum(
                all_r[:], all_acc[:].rearrange("p (r g) -> p r g", g=ng),
                axis=mybir.AxisListType.X)

            valid = cpool.tile([128, rt], F32)
            p1 = cpool.tile([128, rt], F32)
            la = cpool.tile([128, rt], F32)
            lp = cpool.tile([128, rt], F32)
            contrib = cpool.tile([128, rt], F32)
            cv = cpool.tile([128, 2], F32)
            ones = cpool.tile([128, 1], F32)
            tot_sb = cpool.tile([1, 2], F32)

            nc.vector.tensor_scalar(out=valid[:], in0=pos_r[:], scalar1=0.0,
                                    scalar2=None, op0=mybir.AluOpType.is_gt)
            # p1 = pos + (pos <= 0)  -> pos for valid rows, 1 for invalid
            nc.vector.scalar_tensor_tensor(
                out=p1[:], in0=pos_r[:], scalar=0.0, in1=pos_r[:],
                op0=mybir.AluOpType.is_le, op1=mybir.AluOpType.add)
            nc.scalar.activation(la[:], all_r[:],
                                 mybir.ActivationFunctionType.Ln, bias=eps_t[:])
            nc.scalar.activation(lp[:], p1[:],
                                 mybir.ActivationFunctionType.Ln)
            # contrib = valid*la - lp   (lp = 0 for invalid rows)
            nc.vector.tensor_mul(contrib[:], valid[:], la[:])
            nc.vector.tensor_sub(contrib[:], contrib[:], lp[:])

            nc.vector.reduce_sum(cv[:, 0:1], contrib[:], axis=mybir.AxisListType.X)
            nc.vector.reduce_sum(cv[:, 1:2], valid[:], axis=mybir.AxisListType.X)

            nc.vector.memset(ones[:], 1.0)
            tot_ps = psA.tile([1, 2], F32, tag="sim")
            nc.tensor.matmul(tot_ps[:], ones[:], cv[:], start=True, stop=True)
            nc.vector.tensor_copy(tot_sb[:], tot_ps[:])
            nc.sync.dma_start(out_d[:], tot_sb[:])
            if debug_outputs:
                nc.sync.dma_start(posr_d[:], pos_r[:])
                nc.sync.dma_start(allr_d[:], all_r[:])

    nc.compile()
    return nc


_CACHE = {}


def _get_nc(b=B, n_cores=N_CORES):
    key = (b, n_cores)
    if key not in _CACHE:
        _CACHE[key] = build(b, n_cores)
    return _CACHE[key]


def make_in_maps(embeddings, labels, b=B, n_cores=N_CORES):
    embT = np.ascontiguousarray(
        embeddings.astype(np.float32).T).astype(ml_dtypes.bfloat16)  # [D, b]
    labT = np.full((32, b), -1.0, dtype=np.float32)
    labT[:A] = np.asarray(labels).astype(np.float32).T
    labT = labT.astype(ml_dtypes.bfloat16)
    ident = np.eye(128, dtype=np.float32).astype(ml_dtypes.bfloat16)
    dneg = (np.eye(128, dtype=np.float32) * NEG).astype(ml_dtypes.bfloat16)
    aug = np.empty((2, b), dtype=np.float32)
    aug[0] = -(A / 2.0 - 0.5)
    aug[1] = 1.0
    aug = aug.astype(ml_dtypes.bfloat16)
    rpc = b // n_cores
    in_maps = []
    for c in range(n_cores):
        s = c * rpc
        emb_rot = np.ascontiguousarray(
            np.concatenate([embT[:, s:], embT[:, :s]], axis=1))
        lab_rot = np.ascontiguousarray(
            np.concatenate([labT[:, s:], labT[:, :s]], axis=1))
        in_maps.append({"embT": emb_rot, "labT": lab_rot,
                        "identb": ident, "diagnegb": dneg, "aug": aug})
    return in_maps


def combine_partials(parts):
    """parts: list of [1,2] arrays per core -> scalar loss (reference math)."""
    tot = np.sum(np.stack([p.reshape(2) for p in parts]), axis=0,
                 dtype=np.float64)
    c, v = tot[0], tot[1]
    loss = c / max(v, 1.0) if v > 0 else 0.0
    return np.array(loss, dtype=np.float32)


def kernel(embeddings, labels):
    nc = _get_nc(B, N_CORES)
    in_maps = make_in_maps(embeddings, labels, B, N_CORES)
    res = run_bass_kernel_spmd(nc, in_maps, core_ids=list(range(N_CORES)))
    return combine_partials([r["out"] for r in res.results])
